# revision 25
# baseline (speedup 1.0000x reference)
"""Trainium2 Bass kernel for the neural 2D min-sum LDPC decoder problem.

Strategy (v4)
-------------
Data-parallel over the batch: B=512 codewords, 64 per NeuronCore (8 cores).
Per core, per-edge state lives in SBUF with the graph on the partition axis
and the 64-batch on the free axis (256B rows).

The Tanner graph (edge_v/edge_c) is 6-regular on checks, 3-regular on
variables, built from 3 "layers": sorting each check's edges by edge id
puts exactly one edge of every variable in slots {0,1}, {2,3}, {4,5}.
Variables are relabeled by their slot-{0,1} position, which makes the
layer-0 part of both crossings contiguous.

v4: all SWDGE gathers are issued as PREPARE_ONLY descriptors + per-queue
trigger_dma.  Descriptor generation (the Pool-engine SWDGE ucode, ~3ns
per gathered row -- the machine's scarcest resource here) runs while the
DVE works on the check phase; the triggers fire once the source DRAM
data lands, so only the SDMA transfers remain on the critical path.

The check phase itself never forms x = u - alpha*c2v: the v2c messages
are built in the gather window via the self-cancellation
x_e = llr + alpha*(sum of the OTHER two edges' c2v):
  window:  gm/gh gathers bring c2v of layers 1/2 into variable order;
    x_l0 -> X0 (SBUF, consumed by the next check phase),
    x_l1/x_l2 -> DRAM, routed by the crossing-2 gathers into U slots 2..5.
"""

import sys

for _p in ("/opt/trn_rl_repo",):
    if _p not in sys.path:
        sys.path.insert(0, _p)

import numpy as np

import concourse.bass as bass
import concourse.bacc as bacc
import concourse.mybir as mybir
import concourse.tile as tile
from concourse.bass_utils import run_bass_kernel_spmd

N = 8192          # variable nodes
M = 4096          # check nodes
DC = 6            # check degree (slots)
DV = 3            # variable degree
E = N * DV
B = 512
T = 10
NCORES = 8
BL = B // NCORES  # 64
PB = 128
GB_ = M // PB     # 32 blocks per slot array
CHUNK_BLKS = 4
NCHUNK = GB_ // CHUNK_BLKS

F32 = mybir.dt.float32
I32 = mybir.dt.int32
I16 = mybir.dt.int16
ALU = mybir.AluOpType
ACTF = mybir.ActivationFunctionType


def _derive_graph(edge_v: np.ndarray, edge_c: np.ndarray):
    """Host-side index derivation (layered 6-regular/3-regular graph)."""
    edge_v = np.asarray(edge_v, dtype=np.int64)
    edge_c = np.asarray(edge_c, dtype=np.int64)
    assert edge_v.shape == (E,) and edge_c.shape == (E,)

    order = np.argsort(edge_c, kind="stable")
    assert (edge_c[order] == np.repeat(np.arange(M), DC)).all(), (
        "graph is not 6-regular on checks"
    )
    slot_edge = order.reshape(M, DC).T.copy()  # [DC, M] edge id at (slot j, check c)

    # per-edge position
    j_of_e = np.empty(E, dtype=np.int64)
    c_of_e = np.empty(E, dtype=np.int64)
    for j in range(DC):
        j_of_e[slot_edge[j]] = j
        c_of_e[slot_edge[j]] = np.arange(M)

    # each variable must have exactly one edge in slots {0,1}, {2,3}, {4,5}
    layer_of_e = j_of_e // 2
    ve = np.full((N, 3), -1, dtype=np.int64)
    for lay in range(3):
        sel = np.where(layer_of_e == lay)[0]
        vs = edge_v[sel]
        assert len(np.unique(vs)) == N, f"layer {lay} is not a permutation"
        ve[vs, lay] = sel
    assert (ve >= 0).all()

    # storage row helpers (p-major: row = (c%128)*32 + c//128)
    rowmaj = (c_of_e % PB) * GB_ + (c_of_e // PB)
    # c2v DRAM buffer holds slots 2..5 only
    cdrow = (j_of_e - 2) * M + rowmaj          # valid for slots 2..5
    # u/llr DRAM row of a variable = its slot-{0,1} position
    fr_of_e = j_of_e * M + rowmaj              # valid for slots 0..1
    fr_of_v = fr_of_e[ve[:, 0]]                # [N]

    # x-build gathers (dst = parity pi, list pos = check c): variable at
    # (j=pi, c) -> cdram rows of its layer-1 / layer-2 edges
    ix1 = np.empty((2, M), dtype=np.int16)
    ix2 = np.empty((2, M), dtype=np.int16)
    # crossing-2 gathers (dst slot j=2..5, list pos = c): x DRAM row of v(j,c)
    ixu = np.empty((4, M), dtype=np.int16)
    for pi in range(2):
        e = slot_edge[pi]                      # layer-0 edge at (pi, c)
        v = edge_v[e]
        ix1[pi] = cdrow[ve[v, 1]]
        ix2[pi] = cdrow[ve[v, 2]]
    for j in range(2, DC):
        v = edge_v[slot_edge[j]]
        ixu[j - 2] = fr_of_v[v]

    # host llr/output mapping: variable id at each u/llr DRAM row
    vid_of_fr = np.empty(N, dtype=np.int64)
    vid_of_fr[fr_of_v] = np.arange(N)
    return ix1, ix2, ixu, vid_of_fr


def _wrap_idx(idx_m: np.ndarray) -> np.ndarray:
    """dma_gather index layout: list position k at [k%16, k//16],
    replicated across the 8 groups of 16 partitions."""
    w = idx_m.reshape(M // 16, 16).T
    return np.tile(w, (PB // 16, 1)).copy()


def _build_program(alpha: np.ndarray, beta: np.ndarray) -> bacc.Bacc:
    nc = bacc.Bacc(num_swdge_queues=4)

    llr_t = nc.dram_tensor("llr_t", [N, BL], F32, kind="ExternalInput").ap()
    ix1_d = nc.dram_tensor("ix1", [2, PB, M // 16], I16, kind="ExternalInput").ap()
    ix2_d = nc.dram_tensor("ix2", [2, PB, M // 16], I16, kind="ExternalInput").ap()
    ixu_d = nc.dram_tensor("ixu", [4, PB, M // 16], I16, kind="ExternalInput").ap()
    post_d = nc.dram_tensor("post", [2, PB, GB_, BL], F32, kind="ExternalOutput").ap()
    bits_d = nc.dram_tensor("bits", [2, PB, GB_, BL], I32, kind="ExternalOutput").ap()
    # c2v slots 2..5, ping-pong; x1/x2 (v2c messages of layers 1/2 in
    # variable order), ping-pong
    cdrs = [
        nc.dram_tensor("cda", [4 * M, BL], F32).ap(),
        nc.dram_tensor("cdb", [4 * M, BL], F32).ap(),
    ]
    x1rs = [
        nc.dram_tensor("x1a", [N, BL], F32).ap(),
        nc.dram_tensor("x1b", [N, BL], F32).ap(),
    ]
    x2rs = [
        nc.dram_tensor("x2a", [N, BL], F32).ap(),
        nc.dram_tensor("x2b", [N, BL], F32).ap(),
    ]
    cdrv = [c.rearrange("(j p g) e -> j p g e", j=4, p=PB) for c in cdrs]
    x1rv = [u.rearrange("(pi p g) e -> p pi g e", pi=2, p=PB) for u in x1rs]
    x2rv = [u.rearrange("(pi p g) e -> p pi g e", pi=2, p=PB) for u in x2rs]
    bitv = bits_d.rearrange("pi p g e -> p pi g e")

    # SWDGE slot rotation: every gather (prep or not) advances one slot.
    # queue = slot%4 (strict round-robin keeps the 4 ucode queues busy);
    # DMA-completion sem for preps = sems[slot%8], matching Tile's mod-8
    # DMASW lane rotation so each lane pairs with a stable semaphore.
    SW = [0]
    sems = [nc.alloc_semaphore(f"swdge_dma{i}") for i in range(32)]


    def slot():
        q = SW[0] % 4
        s = sems[SW[0] % 32]
        SW[0] += 1
        return q, s

    S1 = CHUNK_BLKS * BL  # free elems per slot per chunk (256)
    # prepare_only+trigger_dma was tried for both gather phases: desc-gen
    # overlapped the check phase, but the generated synchronization raced on
    # hardware (stale gathers) regardless of sem assignment or explicit
    # barriers -- keep the plain self-firing gather path.
    PREP = False
    PREP2 = False

    with tile.TileContext(nc) as tc:
        with (
            tc.tile_pool(name="persist", bufs=1) as pp,
            tc.tile_pool(name="gbp", bufs=4) as gbp,
            tc.tile_pool(name="xdp", bufs=2) as xdp,
            tc.tile_pool(name="wtp", bufs=1) as wtp,
            tc.tile_pool(name="tmp", bufs=1) as tp,
            tc.tile_pool(name="ps", bufs=1, space="PSUM") as psp,
        ):
            ix1 = [pp.tile([PB, M // 16], I16, tag=f"ix1{i}", name=f"ix1{i}") for i in range(2)]
            ix2 = [pp.tile([PB, M // 16], I16, tag=f"ix2{i}", name=f"ix2{i}") for i in range(2)]
            ixu = [pp.tile([PB, M // 16], I16, tag=f"ixu{i}", name=f"ixu{i}") for i in range(4)]
            for i in range(2):
                nc.sync.dma_start(ix1[i][:], ix1_d[i])
                nc.sync.dma_start(ix2[i][:], ix2_d[i])
            for i in range(4):
                nc.sync.dma_start(ixu[i][:], ixu_d[i])

            # hoisted num_idxs registers (a fresh to_reg per gather costs a
            # Pool MOVE each)
            r256 = nc.gpsimd.to_reg(M // 16)
            r512 = nc.gpsimd.to_reg(M // 8)
            r1024 = nc.gpsimd.to_reg(M // 4)
            r2048 = nc.gpsimd.to_reg(M // 2)

            # llr in variable(-row) order, parity-split: [128, 2, 32, 64]
            LV = pp.tile([PB, 2, GB_, BL], F32, tag="lv", name="lv")
            nc.sync.dma_start(
                LV[:], llr_t.rearrange("(pi p g) e -> p pi g e", pi=2, p=PB)
            )
            # x at positions: slots 0,1 (layer 0, variable order) in X0;
            # slots 2..5 (layers 1/2) gathered into U each iteration
            X0 = pp.tile([PB, 2, GB_, BL], F32, tag="x0", name="x0")
            PRE = pp.tile([PB, 2, GB_, BL], F32, tag="pre", name="pre")
            U = pp.tile([PB, 4, GB_, BL], F32, tag="u", name="u")
            # c2v (all 6 slots, check order)
            C = pp.tile([PB, DC, GB_, BL], F32, tag="c", name="c")

            # t=0: x(0) = llr at every edge
            nc.scalar.activation(X0[:], LV[:], ACTF.Copy)
            for h in range(2):
                for i in range(4):
                    q, _ = slot()
                    nc.gpsimd.dma_gather(
                        U[:, i, h * 16 : (h + 1) * 16, :],
                        llr_t,
                        ixu[i][:, h * 128 : (h + 1) * 128],
                        M // 2, r2048, BL,
                        single_packet=False, queue_num=q,
                    )

            def check_chunk(ck, beta_t, alpha_t, cdvt, last):
                """min-sum check update for chunk ck (CHUNK_BLKS blocks).
                Consumes X0 (slots 0,1) + U (slots 2..5); writes C and DMAs
                slots 2..5 to DRAM."""
                b0 = ck * CHUNK_BLKS
                bs = slice(b0, b0 + CHUNK_BLKS)
                mg = tp.tile([PB, DC, CHUNK_BLKS, BL], F32, tag="mg", name="mg")
                sg = tp.tile([PB, DC, CHUNK_BLKS, BL], F32, tag="sg", name="sg")
                nc.scalar.activation(mg[:, 0:2], X0[:, :, bs, :], ACTF.Abs)
                nc.scalar.activation(mg[:, 2:6], U[:, :, bs, :], ACTF.Abs)
                nc.scalar.activation(sg[:, 0:2], X0[:, :, bs, :], ACTF.Sign)
                nc.scalar.activation(sg[:, 2:6], U[:, :, bs, :], ACTF.Sign)
                pp3 = tp.tile([PB, 3, CHUNK_BLKS, BL], F32, tag="pp3", name="pp3")
                sp3 = tp.tile([PB, 3, CHUNK_BLKS, BL], F32, tag="sp3", name="sp3")
                nc.vector.tensor_tensor(pp3[:], mg[:, 0::2], mg[:, 1::2], ALU.min)
                nc.vector.tensor_tensor(sp3[:], sg[:, 0::2], sg[:, 1::2], ALU.mult)
                # leave-one-pair-out mins
                qq = psp.tile([PB, 3, CHUNK_BLKS, BL], F32, tag="qq", name="qq")
                nc.vector.tensor_tensor(qq[:, 0], pp3[:, 1], pp3[:, 2], ALU.min)
                pv = pp3[:]
                pswap = bass.AP(
                    pv.tensor, pv.offset + 2 * S1,
                    [pv.ap[0], [-S1, 2], [1, S1]],
                )
                p0b = (pp3[:, 0].rearrange("p b e -> p (b e)")[:, None, :]
                       .to_broadcast([PB, 2, S1]))
                nc.vector.tensor_tensor(
                    qq[:, 1:3].rearrange("p a b e -> p a (b e)"), pswap, p0b, ALU.min
                )
                # total sign product * beta
                bsp = psp.tile([PB, CHUNK_BLKS, BL], F32, tag="bsp", name="bsp")
                nc.vector.tensor_tensor(bsp[:], sp3[:, 0], sp3[:, 1], ALU.mult)
                nc.vector.scalar_tensor_tensor(
                    bsp[:], bsp[:], float(beta_t), sp3[:, 2], ALU.mult, ALU.mult
                )
                # leave-one-out min: ex[j] = min(mg[partner(j)], qq[j//2])
                ex = psp.tile([PB, DC, CHUNK_BLKS, BL], F32, tag="ex", name="ex")
                mv = mg[:]
                msw = bass.AP(
                    mv.tensor, mv.offset + S1,
                    [mv.ap[0], [2 * S1, 3], [-S1, 2], [1, S1]],
                )
                qb = (qq[:].rearrange("p a b e -> p a (b e)")[:, :, None, :]
                      .to_broadcast([PB, 3, 2, S1]))
                nc.vector.tensor_tensor(
                    ex[:].rearrange("p (a b) c e -> p a b (c e)", a=3), msw, qb, ALU.min
                )
                # c2v = (sign * beta*sprod) * exclmin
                bb = bsp[:, None, :, :].to_broadcast([PB, DC, CHUNK_BLKS, BL])
                nc.vector.tensor_tensor(sg[:], sg[:], bb, ALU.mult)
                nc.vector.tensor_tensor(C[:, :, bs, :], sg[:], ex[:], ALU.mult)
                for j in range(2, DC):
                    nc.sync.dma_start(cdvt[j - 2][:, bs, :], C[:, j, bs, :])

            for t in range(T):
                beta_t = float(beta[t])
                alpha_t = float(alpha[t])
                cdt, cdvt = cdrs[t % 2], cdrv[t % 2]
                cdvt_full = cdvt
                x1t, x1vt = x1rs[t % 2], x1rv[t % 2]
                x2t, x2vt = x2rs[t % 2], x2rv[t % 2]
                last = t == T - 1

                # --- check phase (DVE/ACT; Pool desc-gens the preps below) ---
                for ck in range(NCHUNK):
                    check_chunk(ck, beta_t, alpha_t, cdvt, last)

                # --- gm/gh preps: c2v of layers 1/2 -> variable order.
                # desc-gen runs during the check phase; the triggers wait for
                # the c2v DRAM writes. ---
                gms, ghs = [], []
                for h in range(4):
                    ls = slice(h * 64, (h + 1) * 64)
                    gm = gbp.tile([PB, 2, 8, BL], F32, tag="gm", name=f"gm{t}_{h}")
                    gh = gbp.tile([PB, 2, 8, BL], F32, tag="gh", name=f"gh{t}_{h}")
                    gms.append(gm)
                    ghs.append(gh)
                    for pi in range(2):
                        q, s = slot()
                        nc.gpsimd.dma_gather(
                            gh[:, pi], cdt, ix2[pi][:, ls], M // 4, r1024, BL,
                            single_packet=False, queue_num=q,
                            prepare_only=PREP, sem=s if PREP else None,
                        )
                    for pi in range(2):
                        q, s = slot()
                        nc.gpsimd.dma_gather(
                            gm[:, pi], cdt, ix1[pi][:, ls], M // 4, r1024, BL,
                            single_packet=False, queue_num=q,
                            prepare_only=PREP, sem=s if PREP else None,
                        )
                if PREP:
                    # barrier: echo-read one element from every c2v chunk
                    # write's range, then a Pool op consuming the echo tile
                    # -- the triggers (in-order on Pool) thereby fire only
                    # after all 32 c2v writes have landed in DRAM.
                    echo = tp.tile([PB, 4, 8, 1], F32, tag="echo", name=f"echo{t}")
                    for j in range(4):
                        nc.sync.dma_start(
                            echo[:, j], cdvt_full[j][:, ::CHUNK_BLKS, 0:1]
                        )
                    scr = tp.tile([PB, 32], F32, tag="scr", name=f"scr{t}")
                    nc.gpsimd.partition_broadcast(
                        scr[:], echo[:].rearrange("p j g e -> p (j g e)")
                    )
                    for q in range(4):
                        nc.gpsimd.trigger_dma(
                            count=None, queue_num=q,
                            signals_writable=[scr[:, q : q + 1]],
                        )

                # pre = llr + alpha*C0 (gather-independent): the DVE does
                # this while the first gm/gh gathers are still in flight
                if not last:
                    nc.vector.scalar_tensor_tensor(
                        PRE[:], C[:, 0:2, :, :], alpha_t, LV[:], ALU.mult, ALU.add
                    )

                # --- window: x build (or posterior on the last iteration) ---
                for h in range(4):
                    hs = slice(h * 8, (h + 1) * 8)
                    gm, gh = gms[h], ghs[h]
                    lvh = LV[:, :, hs, :]
                    c0h = C[:, 0:2, hs, :]
                    w1 = wtp.tile([PB, 2, 8, BL], F32, tag="w1", name="w1")
                    if last:
                        # posterior = llr + (C0 + gm + gh); bits = post < 0
                        nc.vector.tensor_tensor(w1[:], gm[:], gh[:], ALU.add)
                        nc.vector.tensor_tensor(w1[:], w1[:], c0h, ALU.add)
                        nc.vector.tensor_tensor(w1[:], w1[:], lvh, ALU.add)
                        bt = xdp.tile([PB, 2, 8, BL], I32, tag="xd1", name="bt")
                        nc.vector.tensor_scalar(bt[:], w1[:], 0.0, None, ALU.is_lt)
                        for pi in range(2):
                            nc.sync.dma_start(post_d[pi][:, hs, :], w1[:, pi])
                        nc.sync.dma_start(bitv[:, :, hs, :], bt[:])
                    else:
                        # x_l1 = pre + a*gh -> x1 DRAM (fires after the gh
                        # pair, overlapping the gm gathers)
                        # x_l2 = pre + a*gm -> x2 DRAM
                        # x_l0 = llr + a*(gm+gh) -> X0 (next check phase)
                        xd1 = xdp.tile([PB, 2, 8, BL], F32, tag="xd1", name="xd1")
                        xd2 = xdp.tile([PB, 2, 8, BL], F32, tag="xd2", name="xd2")
                        # per-parity ops: each fires as soon as its single
                        # gather lands instead of waiting for the pair
                        for pi in range(2):
                            nc.vector.scalar_tensor_tensor(
                                xd1[:, pi], gh[:, pi], alpha_t,
                                PRE[:, pi, hs, :], ALU.mult, ALU.add
                            )
                            nc.sync.dma_start(x1vt[:, pi, hs, :], xd1[:, pi])
                        for pi in range(2):
                            nc.vector.scalar_tensor_tensor(
                                xd2[:, pi], gm[:, pi], alpha_t,
                                PRE[:, pi, hs, :], ALU.mult, ALU.add
                            )
                            nc.sync.dma_start(x2vt[:, pi, hs, :], xd2[:, pi])
                        for pi in range(2):
                            nc.vector.tensor_tensor(
                                w1[:, pi], gm[:, pi], gh[:, pi], ALU.add
                            )
                            nc.vector.scalar_tensor_tensor(
                                X0[:, pi, hs, :], w1[:, pi], alpha_t,
                                LV[:, pi, hs, :], ALU.mult, ALU.add
                            )

                if not last:
                    # --- crossing 2 preps: x -> position order, slots 2..5.
                    # desc-gen overlaps the gm/gh transfers + window math;
                    # triggers wait for the x DRAM writes. ---
                    for b0, nb in ((0, 4), (4, 4), (8, 8), (16, 8), (24, 8)):
                        gs = slice(b0 * 8, (b0 + nb) * 8)
                        ds = slice(b0, b0 + nb)
                        nreg = r512 if nb == 4 else r1024
                        for i in range(4):
                            q, s = slot()
                            nc.gpsimd.dma_gather(
                                U[:, i, ds, :],
                                x1t if i < 2 else x2t,
                                ixu[i][:, gs],
                                nb * PB, nreg, BL,
                                single_packet=False, queue_num=q,
                                prepare_only=PREP2, sem=s if PREP2 else None,
                            )
                    if PREP2:
                        echo2 = tp.tile([PB, 2, 2, 8, 1], F32, tag="echo2", name=f"echo2_{t}")
                        for pi in range(2):
                            nc.sync.dma_start(
                                echo2[:, 0, pi], x1vt[:, pi, ::CHUNK_BLKS, 0:1]
                            )
                            nc.sync.dma_start(
                                echo2[:, 1, pi], x2vt[:, pi, ::CHUNK_BLKS, 0:1]
                            )
                        scr2 = tp.tile([PB, 32], F32, tag="scr", name=f"scr2_{t}")
                        nc.gpsimd.partition_broadcast(
                            scr2[:], echo2[:].rearrange("p a b g e -> p (a b g e)")
                        )
                        for q in range(4):
                            nc.gpsimd.trigger_dma(
                                count=None, queue_num=q,
                                signals_writable=[scr2[:, q : q + 1]],
                            )

    nc.compile()
    return nc


def _prepare(llr, edge_v, edge_c, beta, alpha):
    ix1, ix2, ixu, vid_of_fr = _derive_graph(edge_v, edge_c)
    ix1w = np.stack([_wrap_idx(ix1[i]) for i in range(2)])
    ix2w = np.stack([_wrap_idx(ix2[i]) for i in range(2)])
    ixuw = np.stack([_wrap_idx(ixu[i]) for i in range(4)])

    llr = np.asarray(llr, dtype=np.float32)
    in_maps = []
    for k in range(NCORES):
        llr_t = np.ascontiguousarray(llr[k * BL : (k + 1) * BL, vid_of_fr].T)
        in_maps.append({"llr_t": llr_t, "ix1": ix1w, "ix2": ix2w, "ixu": ixuw})
    return in_maps, vid_of_fr


def _assemble(results, vid_of_fr):
    posterior = np.empty((B, N), dtype=np.float32)
    bits = np.empty((B, N), dtype=np.int32)
    for k in range(NCORES):
        pd = results[k]["post"].reshape(N, BL)  # row = pi*4096 + p*32 + g
        bd = results[k]["bits"].reshape(N, BL)
        posterior[k * BL : (k + 1) * BL, vid_of_fr] = pd.T
        bits[k * BL : (k + 1) * BL, vid_of_fr] = bd.T
    return bits, posterior


def _run(llr, edge_v, edge_c, beta, alpha, trace=False, tmpdir=None):
    in_maps, vid_of_fr = _prepare(llr, edge_v, edge_c, beta, alpha)
    nc = _build_program(np.asarray(alpha, np.float32), np.asarray(beta, np.float32))
    res = run_bass_kernel_spmd(
        nc, in_maps, list(range(NCORES)), trace=trace, tmpdir=tmpdir
    )
    return _assemble(res.results, vid_of_fr), res


def kernel(llr, edge_v, edge_c, beta, alpha):
    (bits, posterior), _ = _run(llr, edge_v, edge_c, beta, alpha, trace=False)
    return bits, posterior


# revision 26
# speedup vs baseline: 1.0083x; 1.0083x over previous
"""Trainium2 Bass kernel for the neural 2D min-sum LDPC decoder problem.

Strategy (v4)
-------------
Data-parallel over the batch: B=512 codewords, 64 per NeuronCore (8 cores).
Per core, per-edge state lives in SBUF with the graph on the partition axis
and the 64-batch on the free axis (256B rows).

The Tanner graph (edge_v/edge_c) is 6-regular on checks, 3-regular on
variables, built from 3 "layers": sorting each check's edges by edge id
puts exactly one edge of every variable in slots {0,1}, {2,3}, {4,5}.
Variables are relabeled by their slot-{0,1} position, which makes the
layer-0 part of both crossings contiguous.

v4: all SWDGE gathers are issued as PREPARE_ONLY descriptors + per-queue
trigger_dma.  Descriptor generation (the Pool-engine SWDGE ucode, ~3ns
per gathered row -- the machine's scarcest resource here) runs while the
DVE works on the check phase; the triggers fire once the source DRAM
data lands, so only the SDMA transfers remain on the critical path.

The check phase itself never forms x = u - alpha*c2v: the v2c messages
are built in the gather window via the self-cancellation
x_e = llr + alpha*(sum of the OTHER two edges' c2v):
  window:  gm/gh gathers bring c2v of layers 1/2 into variable order;
    x_l0 -> X0 (SBUF, consumed by the next check phase),
    x_l1/x_l2 -> DRAM, routed by the crossing-2 gathers into U slots 2..5.
"""

import sys

for _p in ("/opt/trn_rl_repo",):
    if _p not in sys.path:
        sys.path.insert(0, _p)

import numpy as np

import concourse.bass as bass
import concourse.bacc as bacc
import concourse.mybir as mybir
import concourse.tile as tile
from concourse.bass_utils import run_bass_kernel_spmd

N = 8192          # variable nodes
M = 4096          # check nodes
DC = 6            # check degree (slots)
DV = 3            # variable degree
E = N * DV
B = 512
T = 10
NCORES = 8
BL = B // NCORES  # 64
PB = 128
GB_ = M // PB     # 32 blocks per slot array
CHUNK_BLKS = 4
NCHUNK = GB_ // CHUNK_BLKS

F32 = mybir.dt.float32
I32 = mybir.dt.int32
I16 = mybir.dt.int16
ALU = mybir.AluOpType
ACTF = mybir.ActivationFunctionType


def _derive_graph(edge_v: np.ndarray, edge_c: np.ndarray):
    """Host-side index derivation (layered 6-regular/3-regular graph)."""
    edge_v = np.asarray(edge_v, dtype=np.int64)
    edge_c = np.asarray(edge_c, dtype=np.int64)
    assert edge_v.shape == (E,) and edge_c.shape == (E,)

    order = np.argsort(edge_c, kind="stable")
    assert (edge_c[order] == np.repeat(np.arange(M), DC)).all(), (
        "graph is not 6-regular on checks"
    )
    slot_edge = order.reshape(M, DC).T.copy()  # [DC, M] edge id at (slot j, check c)

    # per-edge position
    j_of_e = np.empty(E, dtype=np.int64)
    c_of_e = np.empty(E, dtype=np.int64)
    for j in range(DC):
        j_of_e[slot_edge[j]] = j
        c_of_e[slot_edge[j]] = np.arange(M)

    # each variable must have exactly one edge in slots {0,1}, {2,3}, {4,5}
    layer_of_e = j_of_e // 2
    ve = np.full((N, 3), -1, dtype=np.int64)
    for lay in range(3):
        sel = np.where(layer_of_e == lay)[0]
        vs = edge_v[sel]
        assert len(np.unique(vs)) == N, f"layer {lay} is not a permutation"
        ve[vs, lay] = sel
    assert (ve >= 0).all()

    # storage row helpers (p-major: row = (c%128)*32 + c//128)
    rowmaj = (c_of_e % PB) * GB_ + (c_of_e // PB)
    # c2v DRAM buffer holds slots 2..5 only
    cdrow = (j_of_e - 2) * M + rowmaj          # valid for slots 2..5
    # u/llr DRAM row of a variable = its slot-{0,1} position
    fr_of_e = j_of_e * M + rowmaj              # valid for slots 0..1
    fr_of_v = fr_of_e[ve[:, 0]]                # [N]

    # x-build gathers (dst = parity pi, list pos = check c): variable at
    # (j=pi, c) -> cdram rows of its layer-1 / layer-2 edges
    ix1 = np.empty((2, M), dtype=np.int16)
    ix2 = np.empty((2, M), dtype=np.int16)
    # crossing-2 gathers (dst slot j=2..5, list pos = c): x DRAM row of v(j,c)
    ixu = np.empty((4, M), dtype=np.int16)
    for pi in range(2):
        e = slot_edge[pi]                      # layer-0 edge at (pi, c)
        v = edge_v[e]
        ix1[pi] = cdrow[ve[v, 1]]
        ix2[pi] = cdrow[ve[v, 2]]
    for j in range(2, DC):
        v = edge_v[slot_edge[j]]
        ixu[j - 2] = fr_of_v[v]

    # host llr/output mapping: variable id at each u/llr DRAM row
    vid_of_fr = np.empty(N, dtype=np.int64)
    vid_of_fr[fr_of_v] = np.arange(N)
    return ix1, ix2, ixu, vid_of_fr


def _wrap_idx(idx_m: np.ndarray) -> np.ndarray:
    """dma_gather index layout: list position k at [k%16, k//16],
    replicated across the 8 groups of 16 partitions."""
    w = idx_m.reshape(M // 16, 16).T
    return np.tile(w, (PB // 16, 1)).copy()


def _build_program(alpha: np.ndarray, beta: np.ndarray) -> bacc.Bacc:
    nc = bacc.Bacc(num_swdge_queues=4)

    llr_t = nc.dram_tensor("llr_t", [N, BL], F32, kind="ExternalInput").ap()
    ix1_d = nc.dram_tensor("ix1", [2, PB, M // 16], I16, kind="ExternalInput").ap()
    ix2_d = nc.dram_tensor("ix2", [2, PB, M // 16], I16, kind="ExternalInput").ap()
    ixu_d = nc.dram_tensor("ixu", [4, PB, M // 16], I16, kind="ExternalInput").ap()
    post_d = nc.dram_tensor("post", [2, PB, GB_, BL], F32, kind="ExternalOutput").ap()
    bits_d = nc.dram_tensor("bits", [2, PB, GB_, BL], I32, kind="ExternalOutput").ap()
    # c2v slots 2..5, ping-pong; x1/x2 (v2c messages of layers 1/2 in
    # variable order), ping-pong
    cdrs = [
        nc.dram_tensor("cda", [4 * M, BL], F32).ap(),
        nc.dram_tensor("cdb", [4 * M, BL], F32).ap(),
    ]
    x1rs = [
        nc.dram_tensor("x1a", [N, BL], F32).ap(),
        nc.dram_tensor("x1b", [N, BL], F32).ap(),
    ]
    x2rs = [
        nc.dram_tensor("x2a", [N, BL], F32).ap(),
        nc.dram_tensor("x2b", [N, BL], F32).ap(),
    ]
    cdrv = [c.rearrange("(j p g) e -> j p g e", j=4, p=PB) for c in cdrs]
    x1rv = [u.rearrange("(pi p g) e -> p pi g e", pi=2, p=PB) for u in x1rs]
    x2rv = [u.rearrange("(pi p g) e -> p pi g e", pi=2, p=PB) for u in x2rs]
    bitv = bits_d.rearrange("pi p g e -> p pi g e")

    # SWDGE slot rotation: every gather (prep or not) advances one slot.
    # queue = slot%4 (strict round-robin keeps the 4 ucode queues busy);
    # DMA-completion sem for preps = sems[slot%8], matching Tile's mod-8
    # DMASW lane rotation so each lane pairs with a stable semaphore.
    SW = [0]
    sems = [nc.alloc_semaphore(f"swdge_dma{i}") for i in range(32)]


    def slot():
        q = SW[0] % 4
        s = sems[SW[0] % 32]
        SW[0] += 1
        return q, s

    S1 = CHUNK_BLKS * BL  # free elems per slot per chunk (256)
    # prepare_only+trigger_dma was tried for both gather phases: desc-gen
    # overlapped the check phase, but the generated synchronization raced on
    # hardware (stale gathers) regardless of sem assignment or explicit
    # barriers -- keep the plain self-firing gather path.
    PREP = False
    PREP2 = False

    with tile.TileContext(nc) as tc:
        with (
            tc.tile_pool(name="persist", bufs=1) as pp,
            tc.tile_pool(name="gbp", bufs=4) as gbp,
            tc.tile_pool(name="xdp", bufs=2) as xdp,
            tc.tile_pool(name="wtp", bufs=1) as wtp,
            tc.tile_pool(name="tmp", bufs=1) as tp,
            tc.tile_pool(name="ps", bufs=1, space="PSUM") as psp,
        ):
            ix1 = [pp.tile([PB, M // 16], I16, tag=f"ix1{i}", name=f"ix1{i}") for i in range(2)]
            ix2 = [pp.tile([PB, M // 16], I16, tag=f"ix2{i}", name=f"ix2{i}") for i in range(2)]
            ixu = [pp.tile([PB, M // 16], I16, tag=f"ixu{i}", name=f"ixu{i}") for i in range(4)]
            for i in range(2):
                nc.sync.dma_start(ix1[i][:], ix1_d[i])
                nc.sync.dma_start(ix2[i][:], ix2_d[i])
            for i in range(4):
                nc.sync.dma_start(ixu[i][:], ixu_d[i])

            # hoisted num_idxs registers (a fresh to_reg per gather costs a
            # Pool MOVE each)
            r256 = nc.gpsimd.to_reg(M // 16)
            r512 = nc.gpsimd.to_reg(M // 8)
            r1024 = nc.gpsimd.to_reg(M // 4)
            r2048 = nc.gpsimd.to_reg(M // 2)

            # llr in variable(-row) order, parity-split: [128, 2, 32, 64]
            LV = pp.tile([PB, 2, GB_, BL], F32, tag="lv", name="lv")
            nc.sync.dma_start(
                LV[:], llr_t.rearrange("(pi p g) e -> p pi g e", pi=2, p=PB)
            )
            # x at positions: slots 0,1 (layer 0, variable order) in X0;
            # slots 2..5 (layers 1/2) gathered into U each iteration
            X0 = pp.tile([PB, 2, GB_, BL], F32, tag="x0", name="x0")
            PRE = pp.tile([PB, 2, GB_, BL], F32, tag="pre", name="pre")
            U = pp.tile([PB, 4, GB_, BL], F32, tag="u", name="u")
            # c2v (all 6 slots, check order)
            C = pp.tile([PB, DC, GB_, BL], F32, tag="c", name="c")

            # t=0: x(0) = llr at every edge
            nc.scalar.activation(X0[:], LV[:], ACTF.Copy)
            for h in range(2):
                for i in range(4):
                    q, _ = slot()
                    nc.gpsimd.dma_gather(
                        U[:, i, h * 16 : (h + 1) * 16, :],
                        llr_t,
                        ixu[i][:, h * 128 : (h + 1) * 128],
                        M // 2, r2048, BL,
                        single_packet=False, queue_num=q,
                    )

            def check_chunk(ck, beta_t, alpha_t, cdvt, last):
                """min-sum check update for chunk ck (CHUNK_BLKS blocks).
                Consumes X0 (slots 0,1) + U (slots 2..5); writes C and DMAs
                slots 2..5 to DRAM."""
                b0 = ck * CHUNK_BLKS
                bs = slice(b0, b0 + CHUNK_BLKS)
                mg = tp.tile([PB, DC, CHUNK_BLKS, BL], F32, tag="mg", name="mg")
                sg = tp.tile([PB, DC, CHUNK_BLKS, BL], F32, tag="sg", name="sg")
                nc.scalar.activation(mg[:, 0:2], X0[:, :, bs, :], ACTF.Abs)
                nc.scalar.activation(mg[:, 2:6], U[:, :, bs, :], ACTF.Abs)
                nc.scalar.activation(sg[:, 0:2], X0[:, :, bs, :], ACTF.Sign)
                nc.scalar.activation(sg[:, 2:6], U[:, :, bs, :], ACTF.Sign)
                pp3 = tp.tile([PB, 3, CHUNK_BLKS, BL], F32, tag="pp3", name="pp3")
                sp3 = tp.tile([PB, 3, CHUNK_BLKS, BL], F32, tag="sp3", name="sp3")
                nc.vector.tensor_tensor(pp3[:], mg[:, 0::2], mg[:, 1::2], ALU.min)
                nc.vector.tensor_tensor(sp3[:], sg[:, 0::2], sg[:, 1::2], ALU.mult)
                # leave-one-pair-out mins
                qq = psp.tile([PB, 3, CHUNK_BLKS, BL], F32, tag="qq", name="qq")
                nc.vector.tensor_tensor(qq[:, 0], pp3[:, 1], pp3[:, 2], ALU.min)
                pv = pp3[:]
                pswap = bass.AP(
                    pv.tensor, pv.offset + 2 * S1,
                    [pv.ap[0], [-S1, 2], [1, S1]],
                )
                p0b = (pp3[:, 0].rearrange("p b e -> p (b e)")[:, None, :]
                       .to_broadcast([PB, 2, S1]))
                nc.vector.tensor_tensor(
                    qq[:, 1:3].rearrange("p a b e -> p a (b e)"), pswap, p0b, ALU.min
                )
                # total sign product * beta
                bsp = psp.tile([PB, CHUNK_BLKS, BL], F32, tag="bsp", name="bsp")
                nc.vector.tensor_tensor(bsp[:], sp3[:, 0], sp3[:, 1], ALU.mult)
                nc.vector.scalar_tensor_tensor(
                    bsp[:], bsp[:], float(beta_t), sp3[:, 2], ALU.mult, ALU.mult
                )
                # leave-one-out min: ex[j] = min(mg[partner(j)], qq[j//2])
                ex = psp.tile([PB, DC, CHUNK_BLKS, BL], F32, tag="ex", name="ex")
                mv = mg[:]
                msw = bass.AP(
                    mv.tensor, mv.offset + S1,
                    [mv.ap[0], [2 * S1, 3], [-S1, 2], [1, S1]],
                )
                qb = (qq[:].rearrange("p a b e -> p a (b e)")[:, :, None, :]
                      .to_broadcast([PB, 3, 2, S1]))
                nc.vector.tensor_tensor(
                    ex[:].rearrange("p (a b) c e -> p a b (c e)", a=3), msw, qb, ALU.min
                )
                # c2v = (sign * beta*sprod) * exclmin
                bb = bsp[:, None, :, :].to_broadcast([PB, DC, CHUNK_BLKS, BL])
                nc.vector.tensor_tensor(sg[:], sg[:], bb, ALU.mult)
                nc.vector.tensor_tensor(C[:, :, bs, :], sg[:], ex[:], ALU.mult)
                for j in range(2, DC):
                    nc.sync.dma_start(cdvt[j - 2][:, bs, :], C[:, j, bs, :])

            for t in range(T):
                beta_t = float(beta[t])
                alpha_t = float(alpha[t])
                cdt, cdvt = cdrs[t % 2], cdrv[t % 2]
                cdvt_full = cdvt
                x1t, x1vt = x1rs[t % 2], x1rv[t % 2]
                x2t, x2vt = x2rs[t % 2], x2rv[t % 2]
                last = t == T - 1

                # --- check phase (DVE/ACT; Pool desc-gens the preps below) ---
                for ck in range(NCHUNK):
                    check_chunk(ck, beta_t, alpha_t, cdvt, last)

                # --- gm/gh preps: c2v of layers 1/2 -> variable order.
                # desc-gen runs during the check phase; the triggers wait for
                # the c2v DRAM writes. ---
                gms, ghs = [], []
                for h in range(4):
                    ls = slice(h * 64, (h + 1) * 64)
                    gm = gbp.tile([PB, 2, 8, BL], F32, tag="gm", name=f"gm{t}_{h}")
                    gh = gbp.tile([PB, 2, 8, BL], F32, tag="gh", name=f"gh{t}_{h}")
                    gms.append(gm)
                    ghs.append(gh)
                    for pi in range(2):
                        q, s = slot()
                        nc.gpsimd.dma_gather(
                            gh[:, pi], cdt, ix2[pi][:, ls], M // 4, r1024, BL,
                            single_packet=False, queue_num=q,
                            prepare_only=PREP, sem=s if PREP else None,
                        )
                    for pi in range(2):
                        q, s = slot()
                        nc.gpsimd.dma_gather(
                            gm[:, pi], cdt, ix1[pi][:, ls], M // 4, r1024, BL,
                            single_packet=False, queue_num=q,
                            prepare_only=PREP, sem=s if PREP else None,
                        )
                if PREP:
                    # barrier: echo-read one element from every c2v chunk
                    # write's range, then a Pool op consuming the echo tile
                    # -- the triggers (in-order on Pool) thereby fire only
                    # after all 32 c2v writes have landed in DRAM.
                    echo = tp.tile([PB, 4, 8, 1], F32, tag="echo", name=f"echo{t}")
                    for j in range(4):
                        nc.sync.dma_start(
                            echo[:, j], cdvt_full[j][:, ::CHUNK_BLKS, 0:1]
                        )
                    scr = tp.tile([PB, 32], F32, tag="scr", name=f"scr{t}")
                    nc.gpsimd.partition_broadcast(
                        scr[:], echo[:].rearrange("p j g e -> p (j g e)")
                    )
                    for q in range(4):
                        nc.gpsimd.trigger_dma(
                            count=None, queue_num=q,
                            signals_writable=[scr[:, q : q + 1]],
                        )

                # pre = llr + alpha*C0 (gather-independent): the DVE does
                # this while the first gm/gh gathers are still in flight
                if not last:
                    nc.vector.scalar_tensor_tensor(
                        PRE[:], C[:, 0:2, :, :], alpha_t, LV[:], ALU.mult, ALU.add
                    )

                # --- window: x build (or posterior on the last iteration) ---
                for h in range(4):
                    hs = slice(h * 8, (h + 1) * 8)
                    gm, gh = gms[h], ghs[h]
                    lvh = LV[:, :, hs, :]
                    c0h = C[:, 0:2, hs, :]
                    w1 = wtp.tile([PB, 2, 8, BL], F32, tag="w1", name="w1")
                    if last:
                        # posterior = llr + (C0 + gm + gh); bits = post < 0
                        nc.vector.tensor_tensor(w1[:], gm[:], gh[:], ALU.add)
                        nc.vector.tensor_tensor(w1[:], w1[:], c0h, ALU.add)
                        nc.vector.tensor_tensor(w1[:], w1[:], lvh, ALU.add)
                        bt = xdp.tile([PB, 2, 8, BL], I32, tag="xd1", name="bt")
                        nc.vector.tensor_scalar(bt[:], w1[:], 0.0, None, ALU.is_lt)
                        for pi in range(2):
                            nc.sync.dma_start(post_d[pi][:, hs, :], w1[:, pi])
                        nc.sync.dma_start(bitv[:, :, hs, :], bt[:])
                    else:
                        # x_l1 = pre + a*gh -> x1 DRAM (fires after the gh
                        # pair, overlapping the gm gathers)
                        # x_l2 = pre + a*gm -> x2 DRAM
                        # x_l0 = llr + a*(gm+gh) -> X0 (next check phase)
                        xd1 = xdp.tile([PB, 2, 8, BL], F32, tag="xd1", name="xd1")
                        xd2 = xdp.tile([PB, 2, 8, BL], F32, tag="xd2", name="xd2")
                        # per-parity ops: each fires as soon as its single
                        # gather lands instead of waiting for the pair
                        for pi in range(2):
                            nc.vector.scalar_tensor_tensor(
                                xd1[:, pi], gh[:, pi], alpha_t,
                                PRE[:, pi, hs, :], ALU.mult, ALU.add
                            )
                            nc.sync.dma_start(x1vt[:, pi, hs, :], xd1[:, pi])
                        for pi in range(2):
                            nc.vector.scalar_tensor_tensor(
                                xd2[:, pi], gm[:, pi], alpha_t,
                                PRE[:, pi, hs, :], ALU.mult, ALU.add
                            )
                            nc.sync.dma_start(x2vt[:, pi, hs, :], xd2[:, pi])
                        nc.vector.tensor_tensor(w1[:], gm[:], gh[:], ALU.add)
                        nc.vector.scalar_tensor_tensor(
                            X0[:, :, hs, :], w1[:], alpha_t, lvh, ALU.mult, ALU.add
                        )

                if not last:
                    # --- crossing 2 preps: x -> position order, slots 2..5.
                    # desc-gen overlaps the gm/gh transfers + window math;
                    # triggers wait for the x DRAM writes. ---
                    for b0, nb in ((0, 4), (4, 4), (8, 8), (16, 8), (24, 8)):
                        gs = slice(b0 * 8, (b0 + nb) * 8)
                        ds = slice(b0, b0 + nb)
                        nreg = r512 if nb == 4 else r1024
                        for i in range(4):
                            q, s = slot()
                            nc.gpsimd.dma_gather(
                                U[:, i, ds, :],
                                x1t if i < 2 else x2t,
                                ixu[i][:, gs],
                                nb * PB, nreg, BL,
                                single_packet=False, queue_num=q,
                                prepare_only=PREP2, sem=s if PREP2 else None,
                            )
                    if PREP2:
                        echo2 = tp.tile([PB, 2, 2, 8, 1], F32, tag="echo2", name=f"echo2_{t}")
                        for pi in range(2):
                            nc.sync.dma_start(
                                echo2[:, 0, pi], x1vt[:, pi, ::CHUNK_BLKS, 0:1]
                            )
                            nc.sync.dma_start(
                                echo2[:, 1, pi], x2vt[:, pi, ::CHUNK_BLKS, 0:1]
                            )
                        scr2 = tp.tile([PB, 32], F32, tag="scr", name=f"scr2_{t}")
                        nc.gpsimd.partition_broadcast(
                            scr2[:], echo2[:].rearrange("p a b g e -> p (a b g e)")
                        )
                        for q in range(4):
                            nc.gpsimd.trigger_dma(
                                count=None, queue_num=q,
                                signals_writable=[scr2[:, q : q + 1]],
                            )

    nc.compile()
    return nc


def _prepare(llr, edge_v, edge_c, beta, alpha):
    ix1, ix2, ixu, vid_of_fr = _derive_graph(edge_v, edge_c)
    ix1w = np.stack([_wrap_idx(ix1[i]) for i in range(2)])
    ix2w = np.stack([_wrap_idx(ix2[i]) for i in range(2)])
    ixuw = np.stack([_wrap_idx(ixu[i]) for i in range(4)])

    llr = np.asarray(llr, dtype=np.float32)
    in_maps = []
    for k in range(NCORES):
        llr_t = np.ascontiguousarray(llr[k * BL : (k + 1) * BL, vid_of_fr].T)
        in_maps.append({"llr_t": llr_t, "ix1": ix1w, "ix2": ix2w, "ixu": ixuw})
    return in_maps, vid_of_fr


def _assemble(results, vid_of_fr):
    posterior = np.empty((B, N), dtype=np.float32)
    bits = np.empty((B, N), dtype=np.int32)
    for k in range(NCORES):
        pd = results[k]["post"].reshape(N, BL)  # row = pi*4096 + p*32 + g
        bd = results[k]["bits"].reshape(N, BL)
        posterior[k * BL : (k + 1) * BL, vid_of_fr] = pd.T
        bits[k * BL : (k + 1) * BL, vid_of_fr] = bd.T
    return bits, posterior


def _run(llr, edge_v, edge_c, beta, alpha, trace=False, tmpdir=None):
    in_maps, vid_of_fr = _prepare(llr, edge_v, edge_c, beta, alpha)
    nc = _build_program(np.asarray(alpha, np.float32), np.asarray(beta, np.float32))
    res = run_bass_kernel_spmd(
        nc, in_maps, list(range(NCORES)), trace=trace, tmpdir=tmpdir
    )
    return _assemble(res.results, vid_of_fr), res


def kernel(llr, edge_v, edge_c, beta, alpha):
    (bits, posterior), _ = _run(llr, edge_v, edge_c, beta, alpha, trace=False)
    return bits, posterior


# revision 29
# speedup vs baseline: 1.0512x; 1.0426x over previous
"""Trainium2 Bass kernel for the neural 2D min-sum LDPC decoder problem.

Strategy (v4)
-------------
Data-parallel over the batch: B=512 codewords, 64 per NeuronCore (8 cores).
Per core, per-edge state lives in SBUF with the graph on the partition axis
and the 64-batch on the free axis (256B rows).

The Tanner graph (edge_v/edge_c) is 6-regular on checks, 3-regular on
variables, built from 3 "layers": sorting each check's edges by edge id
puts exactly one edge of every variable in slots {0,1}, {2,3}, {4,5}.
Variables are relabeled by their slot-{0,1} position, which makes the
layer-0 part of both crossings contiguous.

v4: all SWDGE gathers are issued as PREPARE_ONLY descriptors + per-queue
trigger_dma.  Descriptor generation (the Pool-engine SWDGE ucode, ~3ns
per gathered row -- the machine's scarcest resource here) runs while the
DVE works on the check phase; the triggers fire once the source DRAM
data lands, so only the SDMA transfers remain on the critical path.

The check phase itself never forms x = u - alpha*c2v: the v2c messages
are built in the gather window via the self-cancellation
x_e = llr + alpha*(sum of the OTHER two edges' c2v):
  window:  gm/gh gathers bring c2v of layers 1/2 into variable order;
    x_l0 -> X0 (SBUF, consumed by the next check phase),
    x_l1/x_l2 -> DRAM, routed by the crossing-2 gathers into U slots 2..5.
"""

import sys

for _p in ("/opt/trn_rl_repo",):
    if _p not in sys.path:
        sys.path.insert(0, _p)

import numpy as np

import concourse.bass as bass
import concourse.bacc as bacc
import concourse.mybir as mybir
import concourse.tile as tile
from concourse.bass_utils import run_bass_kernel_spmd

N = 8192          # variable nodes
M = 4096          # check nodes
DC = 6            # check degree (slots)
DV = 3            # variable degree
E = N * DV
B = 512
T = 10
NCORES = 8
BL = B // NCORES  # 64
PB = 128
GB_ = M // PB     # 32 blocks per slot array
CHUNK_BLKS = 4
NCHUNK = GB_ // CHUNK_BLKS

F32 = mybir.dt.float32
I32 = mybir.dt.int32
I16 = mybir.dt.int16
ALU = mybir.AluOpType
ACTF = mybir.ActivationFunctionType


def _derive_graph(edge_v: np.ndarray, edge_c: np.ndarray):
    """Host-side index derivation (layered 6-regular/3-regular graph)."""
    edge_v = np.asarray(edge_v, dtype=np.int64)
    edge_c = np.asarray(edge_c, dtype=np.int64)
    assert edge_v.shape == (E,) and edge_c.shape == (E,)

    order = np.argsort(edge_c, kind="stable")
    assert (edge_c[order] == np.repeat(np.arange(M), DC)).all(), (
        "graph is not 6-regular on checks"
    )
    slot_edge = order.reshape(M, DC).T.copy()  # [DC, M] edge id at (slot j, check c)

    # per-edge position
    j_of_e = np.empty(E, dtype=np.int64)
    c_of_e = np.empty(E, dtype=np.int64)
    for j in range(DC):
        j_of_e[slot_edge[j]] = j
        c_of_e[slot_edge[j]] = np.arange(M)

    # each variable must have exactly one edge in slots {0,1}, {2,3}, {4,5}
    layer_of_e = j_of_e // 2
    ve = np.full((N, 3), -1, dtype=np.int64)
    for lay in range(3):
        sel = np.where(layer_of_e == lay)[0]
        vs = edge_v[sel]
        assert len(np.unique(vs)) == N, f"layer {lay} is not a permutation"
        ve[vs, lay] = sel
    assert (ve >= 0).all()

    # storage row helpers (p-major: row = (c%128)*32 + c//128)
    rowmaj = (c_of_e % PB) * GB_ + (c_of_e // PB)
    # c2v DRAM buffer holds slots 2..5 only
    cdrow = (j_of_e - 2) * M + rowmaj          # valid for slots 2..5
    # u/llr DRAM row of a variable = its slot-{0,1} position
    fr_of_e = j_of_e * M + rowmaj              # valid for slots 0..1
    fr_of_v = fr_of_e[ve[:, 0]]                # [N]

    # x-build gathers (dst = parity pi, list pos = check c): variable at
    # (j=pi, c) -> cdram rows of its layer-1 / layer-2 edges
    ix1 = np.empty((2, M), dtype=np.int16)
    ix2 = np.empty((2, M), dtype=np.int16)
    # crossing-2 gathers (dst slot j=2..5, list pos = c): x DRAM row of v(j,c)
    ixu = np.empty((4, M), dtype=np.int16)
    for pi in range(2):
        e = slot_edge[pi]                      # layer-0 edge at (pi, c)
        v = edge_v[e]
        ix1[pi] = cdrow[ve[v, 1]]
        ix2[pi] = cdrow[ve[v, 2]]
    for j in range(2, DC):
        v = edge_v[slot_edge[j]]
        ixu[j - 2] = fr_of_v[v]

    # host llr/output mapping: variable id at each u/llr DRAM row
    vid_of_fr = np.empty(N, dtype=np.int64)
    vid_of_fr[fr_of_v] = np.arange(N)
    return ix1, ix2, ixu, vid_of_fr


def _wrap_idx(idx_m: np.ndarray) -> np.ndarray:
    """dma_gather index layout: list position k at [k%16, k//16],
    replicated across the 8 groups of 16 partitions."""
    w = idx_m.reshape(M // 16, 16).T
    return np.tile(w, (PB // 16, 1)).copy()


def _build_program(alpha: np.ndarray, beta: np.ndarray) -> bacc.Bacc:
    nc = bacc.Bacc(num_swdge_queues=4)

    llr_t = nc.dram_tensor("llr_t", [N, BL], F32, kind="ExternalInput").ap()
    ix1_d = nc.dram_tensor("ix1", [2, PB, M // 16], I16, kind="ExternalInput").ap()
    ix2_d = nc.dram_tensor("ix2", [2, PB, M // 16], I16, kind="ExternalInput").ap()
    ixu_d = nc.dram_tensor("ixu", [4, PB, M // 16], I16, kind="ExternalInput").ap()
    post_d = nc.dram_tensor("post", [2, PB, GB_, BL], F32, kind="ExternalOutput").ap()
    bits_d = nc.dram_tensor("bits", [2, PB, GB_, BL], I32, kind="ExternalOutput").ap()
    # c2v slots 2..5, ping-pong; x1/x2 (v2c messages of layers 1/2 in
    # variable order), ping-pong
    cdrs = [
        nc.dram_tensor("cda", [4 * M, BL], F32).ap(),
        nc.dram_tensor("cdb", [4 * M, BL], F32).ap(),
    ]
    x1rs = [
        nc.dram_tensor("x1a", [N, BL], F32).ap(),
        nc.dram_tensor("x1b", [N, BL], F32).ap(),
    ]
    x2rs = [
        nc.dram_tensor("x2a", [N, BL], F32).ap(),
        nc.dram_tensor("x2b", [N, BL], F32).ap(),
    ]
    cdrv = [c.rearrange("(j p g) e -> j p g e", j=4, p=PB) for c in cdrs]
    x1rv = [u.rearrange("(pi p g) e -> p pi g e", pi=2, p=PB) for u in x1rs]
    x2rv = [u.rearrange("(pi p g) e -> p pi g e", pi=2, p=PB) for u in x2rs]
    bitv = bits_d.rearrange("pi p g e -> p pi g e")

    # SWDGE slot rotation: every gather (prep or not) advances one slot.
    # queue = slot%4 (strict round-robin keeps the 4 ucode queues busy);
    # DMA-completion sem for preps = sems[slot%8], matching Tile's mod-8
    # DMASW lane rotation so each lane pairs with a stable semaphore.
    SW = [0]
    sems = [nc.alloc_semaphore(f"swdge_dma{i}") for i in range(32)]


    def slot():
        q = SW[0] % 4
        s = sems[SW[0] % 32]
        SW[0] += 1
        return q, s

    S1 = CHUNK_BLKS * BL  # free elems per slot per chunk (256)
    # prepare_only+trigger_dma was tried for both gather phases: desc-gen
    # overlapped the check phase, but the generated synchronization raced on
    # hardware (stale gathers) regardless of sem assignment or explicit
    # barriers -- keep the plain self-firing gather path.
    PREP = False
    PREP2 = False

    with tile.TileContext(nc) as tc:
        with (
            tc.tile_pool(name="persist", bufs=1) as pp,
            tc.tile_pool(name="gbp", bufs=4) as gbp,
            tc.tile_pool(name="xdp", bufs=2) as xdp,
            tc.tile_pool(name="wtp", bufs=1) as wtp,
            tc.tile_pool(name="tmp", bufs=1) as tp,
            tc.tile_pool(name="ps", bufs=1, space="PSUM") as psp,
        ):
            ix1 = [pp.tile([PB, M // 16], I16, tag=f"ix1{i}", name=f"ix1{i}") for i in range(2)]
            ix2 = [pp.tile([PB, M // 16], I16, tag=f"ix2{i}", name=f"ix2{i}") for i in range(2)]
            ixu = [pp.tile([PB, M // 16], I16, tag=f"ixu{i}", name=f"ixu{i}") for i in range(4)]
            for i in range(2):
                nc.sync.dma_start(ix1[i][:], ix1_d[i])
                nc.sync.dma_start(ix2[i][:], ix2_d[i])
            for i in range(4):
                nc.sync.dma_start(ixu[i][:], ixu_d[i])

            # hoisted num_idxs registers (a fresh to_reg per gather costs a
            # Pool MOVE each)
            r256 = nc.gpsimd.to_reg(M // 16)
            r512 = nc.gpsimd.to_reg(M // 8)
            r1024 = nc.gpsimd.to_reg(M // 4)
            r2048 = nc.gpsimd.to_reg(M // 2)

            # llr in variable(-row) order, parity-split: [128, 2, 32, 64]
            LV = pp.tile([PB, 2, GB_, BL], F32, tag="lv", name="lv")
            nc.sync.dma_start(
                LV[:], llr_t.rearrange("(pi p g) e -> p pi g e", pi=2, p=PB)
            )
            # x at positions: slots 0,1 (layer 0, variable order) in X0;
            # slots 2..5 (layers 1/2) gathered into U each iteration
            X0 = pp.tile([PB, 2, GB_, BL], F32, tag="x0", name="x0")
            PRE = pp.tile([PB, 2, GB_, BL], F32, tag="pre", name="pre")
            U = pp.tile([PB, 4, GB_, BL], F32, tag="u", name="u")
            # c2v (all 6 slots, check order)
            C = pp.tile([PB, DC, GB_, BL], F32, tag="c", name="c")

            # t=0: x(0) = llr at every edge
            nc.scalar.activation(X0[:], LV[:], ACTF.Copy)
            for b0, nb in ((0, 4), (4, 4), (8, 8), (16, 8), (24, 8)):
                nreg = r512 if nb == 4 else r1024
                for i in range(4):
                    q, _ = slot()
                    nc.gpsimd.dma_gather(
                        U[:, i, b0 : b0 + nb, :],
                        llr_t,
                        ixu[i][:, b0 * 8 : (b0 + nb) * 8],
                        nb * PB, nreg, BL,
                        single_packet=False, queue_num=q,
                    )

            def check_chunk(ck, beta_t, alpha_t, cdvt, last):
                """min-sum check update for chunk ck (CHUNK_BLKS blocks).
                Consumes X0 (slots 0,1) + U (slots 2..5); writes C and DMAs
                slots 2..5 to DRAM."""
                b0 = ck * CHUNK_BLKS
                bs = slice(b0, b0 + CHUNK_BLKS)
                mg = tp.tile([PB, DC, CHUNK_BLKS, BL], F32, tag="mg", name="mg")
                sg = tp.tile([PB, DC, CHUNK_BLKS, BL], F32, tag="sg", name="sg")
                nc.scalar.activation(mg[:, 0:2], X0[:, :, bs, :], ACTF.Abs)
                nc.scalar.activation(mg[:, 2:6], U[:, :, bs, :], ACTF.Abs)
                nc.scalar.activation(sg[:, 0:2], X0[:, :, bs, :], ACTF.Sign)
                nc.scalar.activation(sg[:, 2:6], U[:, :, bs, :], ACTF.Sign)
                pp3 = tp.tile([PB, 3, CHUNK_BLKS, BL], F32, tag="pp3", name="pp3")
                sp3 = tp.tile([PB, 3, CHUNK_BLKS, BL], F32, tag="sp3", name="sp3")
                nc.vector.tensor_tensor(pp3[:], mg[:, 0::2], mg[:, 1::2], ALU.min)
                nc.vector.tensor_tensor(sp3[:], sg[:, 0::2], sg[:, 1::2], ALU.mult)
                # leave-one-pair-out mins
                qq = psp.tile([PB, 3, CHUNK_BLKS, BL], F32, tag="qq", name="qq")
                nc.vector.tensor_tensor(qq[:, 0], pp3[:, 1], pp3[:, 2], ALU.min)
                pv = pp3[:]
                pswap = bass.AP(
                    pv.tensor, pv.offset + 2 * S1,
                    [pv.ap[0], [-S1, 2], [1, S1]],
                )
                p0b = (pp3[:, 0].rearrange("p b e -> p (b e)")[:, None, :]
                       .to_broadcast([PB, 2, S1]))
                nc.vector.tensor_tensor(
                    qq[:, 1:3].rearrange("p a b e -> p a (b e)"), pswap, p0b, ALU.min
                )
                # total sign product * beta
                bsp = psp.tile([PB, CHUNK_BLKS, BL], F32, tag="bsp", name="bsp")
                nc.vector.tensor_tensor(bsp[:], sp3[:, 0], sp3[:, 1], ALU.mult)
                nc.vector.scalar_tensor_tensor(
                    bsp[:], bsp[:], float(beta_t), sp3[:, 2], ALU.mult, ALU.mult
                )
                # leave-one-out min: ex[j] = min(mg[partner(j)], qq[j//2])
                ex = psp.tile([PB, DC, CHUNK_BLKS, BL], F32, tag="ex", name="ex")
                mv = mg[:]
                msw = bass.AP(
                    mv.tensor, mv.offset + S1,
                    [mv.ap[0], [2 * S1, 3], [-S1, 2], [1, S1]],
                )
                qb = (qq[:].rearrange("p a b e -> p a (b e)")[:, :, None, :]
                      .to_broadcast([PB, 3, 2, S1]))
                nc.vector.tensor_tensor(
                    ex[:].rearrange("p (a b) c e -> p a b (c e)", a=3), msw, qb, ALU.min
                )
                # c2v = (sign * beta*sprod) * exclmin
                bb = bsp[:, None, :, :].to_broadcast([PB, DC, CHUNK_BLKS, BL])
                nc.vector.tensor_tensor(sg[:], sg[:], bb, ALU.mult)
                nc.vector.tensor_tensor(C[:, :, bs, :], sg[:], ex[:], ALU.mult)
                for j in range(2, DC):
                    nc.sync.dma_start(cdvt[j - 2][:, bs, :], C[:, j, bs, :])

            for t in range(T):
                beta_t = float(beta[t])
                alpha_t = float(alpha[t])
                cdt, cdvt = cdrs[t % 2], cdrv[t % 2]
                cdvt_full = cdvt
                x1t, x1vt = x1rs[t % 2], x1rv[t % 2]
                x2t, x2vt = x2rs[t % 2], x2rv[t % 2]
                last = t == T - 1

                # --- check phase (DVE/ACT; Pool desc-gens the preps below) ---
                for ck in range(NCHUNK):
                    check_chunk(ck, beta_t, alpha_t, cdvt, last)

                # --- gm/gh preps: c2v of layers 1/2 -> variable order.
                # desc-gen runs during the check phase; the triggers wait for
                # the c2v DRAM writes. ---
                # groups (start block, nblocks, tile slot, in-tile offset):
                # the two small trailing groups share tile 3's halves, so 5
                # gather groups fit the 4-deep gm/gh pool without aliasing
                GGRPS = ((0, 8, 0, 0), (8, 8, 1, 0), (16, 8, 2, 0),
                         (24, 4, 3, 0), (28, 4, 3, 4))
                tiles = [
                    (gbp.tile([PB, 2, 8, BL], F32, tag="gm", name=f"gm{t}_{k}"),
                     gbp.tile([PB, 2, 8, BL], F32, tag="gh", name=f"gh{t}_{k}"))
                    for k in range(4)
                ]
                for b0, nb, ti, o in GGRPS:
                    ls = slice(b0 * 8, (b0 + nb) * 8)
                    nreg = r512 if nb == 4 else r1024
                    gm, gh = tiles[ti]
                    for pi in range(2):
                        q, s = slot()
                        nc.gpsimd.dma_gather(
                            gh[:, pi, o : o + nb, :], cdt, ix2[pi][:, ls],
                            nb * PB, nreg, BL,
                            single_packet=False, queue_num=q,
                            prepare_only=PREP, sem=s if PREP else None,
                        )
                    for pi in range(2):
                        q, s = slot()
                        nc.gpsimd.dma_gather(
                            gm[:, pi, o : o + nb, :], cdt, ix1[pi][:, ls],
                            nb * PB, nreg, BL,
                            single_packet=False, queue_num=q,
                            prepare_only=PREP, sem=s if PREP else None,
                        )
                if PREP:
                    # barrier: echo-read one element from every c2v chunk
                    # write's range, then a Pool op consuming the echo tile
                    # -- the triggers (in-order on Pool) thereby fire only
                    # after all 32 c2v writes have landed in DRAM.
                    echo = tp.tile([PB, 4, 8, 1], F32, tag="echo", name=f"echo{t}")
                    for j in range(4):
                        nc.sync.dma_start(
                            echo[:, j], cdvt_full[j][:, ::CHUNK_BLKS, 0:1]
                        )
                    scr = tp.tile([PB, 32], F32, tag="scr", name=f"scr{t}")
                    nc.gpsimd.partition_broadcast(
                        scr[:], echo[:].rearrange("p j g e -> p (j g e)")
                    )
                    for q in range(4):
                        nc.gpsimd.trigger_dma(
                            count=None, queue_num=q,
                            signals_writable=[scr[:, q : q + 1]],
                        )

                # pre = llr + alpha*C0 (gather-independent): the DVE does
                # this while the first gm/gh gathers are still in flight
                if not last:
                    nc.vector.scalar_tensor_tensor(
                        PRE[:], C[:, 0:2, :, :], alpha_t, LV[:], ALU.mult, ALU.add
                    )

                # --- window: x build (or posterior on the last iteration) ---
                for b0, nb, ti, o in GGRPS:
                    hs = slice(b0, b0 + nb)
                    gm = tiles[ti][0][:, :, o : o + nb, :]
                    gh = tiles[ti][1][:, :, o : o + nb, :]
                    lvh = LV[:, :, hs, :]
                    c0h = C[:, 0:2, hs, :]
                    w1f = wtp.tile([PB, 2, 8, BL], F32, tag="w1", name="w1")
                    w1 = w1f[:, :, 0:nb, :]
                    if last:
                        # posterior = llr + (C0 + gm + gh); bits = post < 0
                        nc.vector.tensor_tensor(w1, gm, gh, ALU.add)
                        nc.vector.tensor_tensor(w1, w1, c0h, ALU.add)
                        nc.vector.tensor_tensor(w1, w1, lvh, ALU.add)
                        btf = xdp.tile([PB, 2, 8, BL], I32, tag="xd1", name="bt")
                        nc.vector.tensor_scalar(
                            btf[:, :, 0:nb, :], w1, 0.0, None, ALU.is_lt
                        )
                        for pi in range(2):
                            nc.sync.dma_start(
                                post_d[pi][:, hs, :], w1f[:, pi, 0:nb, :]
                            )
                        nc.sync.dma_start(bitv[:, :, hs, :], btf[:, :, 0:nb, :])
                    else:
                        # x_l1 = pre + a*gh -> x1 DRAM; x_l2 = pre + a*gm -> x2
                        # x_l0 = llr + a*(gm+gh) -> X0 (next check phase)
                        xd1 = xdp.tile([PB, 2, 8, BL], F32, tag="xd1", name="xd1")
                        xd2 = xdp.tile([PB, 2, 8, BL], F32, tag="xd2", name="xd2")
                        # per-parity ops: each fires as soon as its single
                        # gather lands instead of waiting for the pair
                        for pi in range(2):
                            nc.vector.scalar_tensor_tensor(
                                xd1[:, pi, 0:nb, :], gh[:, pi], alpha_t,
                                PRE[:, pi, hs, :], ALU.mult, ALU.add
                            )
                            nc.sync.dma_start(
                                x1vt[:, pi, hs, :], xd1[:, pi, 0:nb, :]
                            )
                        for pi in range(2):
                            nc.vector.scalar_tensor_tensor(
                                xd2[:, pi, 0:nb, :], gm[:, pi], alpha_t,
                                PRE[:, pi, hs, :], ALU.mult, ALU.add
                            )
                            nc.sync.dma_start(
                                x2vt[:, pi, hs, :], xd2[:, pi, 0:nb, :]
                            )
                        nc.vector.tensor_tensor(w1, gm, gh, ALU.add)
                        nc.vector.scalar_tensor_tensor(
                            X0[:, :, hs, :], w1, alpha_t, lvh, ALU.mult, ALU.add
                        )

                if not last:
                    # --- crossing 2 preps: x -> position order, slots 2..5.
                    # desc-gen overlaps the gm/gh transfers + window math;
                    # triggers wait for the x DRAM writes. ---
                    for b0, nb in ((0, 4), (4, 4), (8, 8), (16, 8), (24, 8)):
                        gs = slice(b0 * 8, (b0 + nb) * 8)
                        ds = slice(b0, b0 + nb)
                        nreg = r512 if nb == 4 else r1024
                        for i in range(4):
                            q, s = slot()
                            nc.gpsimd.dma_gather(
                                U[:, i, ds, :],
                                x1t if i < 2 else x2t,
                                ixu[i][:, gs],
                                nb * PB, nreg, BL,
                                single_packet=False, queue_num=q,
                                prepare_only=PREP2, sem=s if PREP2 else None,
                            )
                    if PREP2:
                        echo2 = tp.tile([PB, 2, 2, 8, 1], F32, tag="echo2", name=f"echo2_{t}")
                        for pi in range(2):
                            nc.sync.dma_start(
                                echo2[:, 0, pi], x1vt[:, pi, ::CHUNK_BLKS, 0:1]
                            )
                            nc.sync.dma_start(
                                echo2[:, 1, pi], x2vt[:, pi, ::CHUNK_BLKS, 0:1]
                            )
                        scr2 = tp.tile([PB, 32], F32, tag="scr", name=f"scr2_{t}")
                        nc.gpsimd.partition_broadcast(
                            scr2[:], echo2[:].rearrange("p a b g e -> p (a b g e)")
                        )
                        for q in range(4):
                            nc.gpsimd.trigger_dma(
                                count=None, queue_num=q,
                                signals_writable=[scr2[:, q : q + 1]],
                            )

    nc.compile()
    return nc


def _prepare(llr, edge_v, edge_c, beta, alpha):
    ix1, ix2, ixu, vid_of_fr = _derive_graph(edge_v, edge_c)
    ix1w = np.stack([_wrap_idx(ix1[i]) for i in range(2)])
    ix2w = np.stack([_wrap_idx(ix2[i]) for i in range(2)])
    ixuw = np.stack([_wrap_idx(ixu[i]) for i in range(4)])

    llr = np.asarray(llr, dtype=np.float32)
    in_maps = []
    for k in range(NCORES):
        llr_t = np.ascontiguousarray(llr[k * BL : (k + 1) * BL, vid_of_fr].T)
        in_maps.append({"llr_t": llr_t, "ix1": ix1w, "ix2": ix2w, "ixu": ixuw})
    return in_maps, vid_of_fr


def _assemble(results, vid_of_fr):
    posterior = np.empty((B, N), dtype=np.float32)
    bits = np.empty((B, N), dtype=np.int32)
    for k in range(NCORES):
        pd = results[k]["post"].reshape(N, BL)  # row = pi*4096 + p*32 + g
        bd = results[k]["bits"].reshape(N, BL)
        posterior[k * BL : (k + 1) * BL, vid_of_fr] = pd.T
        bits[k * BL : (k + 1) * BL, vid_of_fr] = bd.T
    return bits, posterior


def _run(llr, edge_v, edge_c, beta, alpha, trace=False, tmpdir=None):
    in_maps, vid_of_fr = _prepare(llr, edge_v, edge_c, beta, alpha)
    nc = _build_program(np.asarray(alpha, np.float32), np.asarray(beta, np.float32))
    res = run_bass_kernel_spmd(
        nc, in_maps, list(range(NCORES)), trace=trace, tmpdir=tmpdir
    )
    return _assemble(res.results, vid_of_fr), res


def kernel(llr, edge_v, edge_c, beta, alpha):
    (bits, posterior), _ = _run(llr, edge_v, edge_c, beta, alpha, trace=False)
    return bits, posterior


# revision 30
# speedup vs baseline: 1.0515x; 1.0002x over previous
"""Trainium2 Bass kernel for the neural 2D min-sum LDPC decoder problem.

Strategy (v4)
-------------
Data-parallel over the batch: B=512 codewords, 64 per NeuronCore (8 cores).
Per core, per-edge state lives in SBUF with the graph on the partition axis
and the 64-batch on the free axis (256B rows).

The Tanner graph (edge_v/edge_c) is 6-regular on checks, 3-regular on
variables, built from 3 "layers": sorting each check's edges by edge id
puts exactly one edge of every variable in slots {0,1}, {2,3}, {4,5}.
Variables are relabeled by their slot-{0,1} position, which makes the
layer-0 part of both crossings contiguous.

v4: all SWDGE gathers are issued as PREPARE_ONLY descriptors + per-queue
trigger_dma.  Descriptor generation (the Pool-engine SWDGE ucode, ~3ns
per gathered row -- the machine's scarcest resource here) runs while the
DVE works on the check phase; the triggers fire once the source DRAM
data lands, so only the SDMA transfers remain on the critical path.

The check phase itself never forms x = u - alpha*c2v: the v2c messages
are built in the gather window via the self-cancellation
x_e = llr + alpha*(sum of the OTHER two edges' c2v):
  window:  gm/gh gathers bring c2v of layers 1/2 into variable order;
    x_l0 -> X0 (SBUF, consumed by the next check phase),
    x_l1/x_l2 -> DRAM, routed by the crossing-2 gathers into U slots 2..5.
"""

import sys

for _p in ("/opt/trn_rl_repo",):
    if _p not in sys.path:
        sys.path.insert(0, _p)

import numpy as np

import concourse.bass as bass
import concourse.bacc as bacc
import concourse.mybir as mybir
import concourse.tile as tile
from concourse.bass_utils import run_bass_kernel_spmd

N = 8192          # variable nodes
M = 4096          # check nodes
DC = 6            # check degree (slots)
DV = 3            # variable degree
E = N * DV
B = 512
T = 10
NCORES = 8
BL = B // NCORES  # 64
PB = 128
GB_ = M // PB     # 32 blocks per slot array
CHUNK_BLKS = 4
NCHUNK = GB_ // CHUNK_BLKS

F32 = mybir.dt.float32
I32 = mybir.dt.int32
I16 = mybir.dt.int16
ALU = mybir.AluOpType
ACTF = mybir.ActivationFunctionType


def _derive_graph(edge_v: np.ndarray, edge_c: np.ndarray):
    """Host-side index derivation (layered 6-regular/3-regular graph)."""
    edge_v = np.asarray(edge_v, dtype=np.int64)
    edge_c = np.asarray(edge_c, dtype=np.int64)
    assert edge_v.shape == (E,) and edge_c.shape == (E,)

    order = np.argsort(edge_c, kind="stable")
    assert (edge_c[order] == np.repeat(np.arange(M), DC)).all(), (
        "graph is not 6-regular on checks"
    )
    slot_edge = order.reshape(M, DC).T.copy()  # [DC, M] edge id at (slot j, check c)

    # per-edge position
    j_of_e = np.empty(E, dtype=np.int64)
    c_of_e = np.empty(E, dtype=np.int64)
    for j in range(DC):
        j_of_e[slot_edge[j]] = j
        c_of_e[slot_edge[j]] = np.arange(M)

    # each variable must have exactly one edge in slots {0,1}, {2,3}, {4,5}
    layer_of_e = j_of_e // 2
    ve = np.full((N, 3), -1, dtype=np.int64)
    for lay in range(3):
        sel = np.where(layer_of_e == lay)[0]
        vs = edge_v[sel]
        assert len(np.unique(vs)) == N, f"layer {lay} is not a permutation"
        ve[vs, lay] = sel
    assert (ve >= 0).all()

    # storage row helpers (p-major: row = (c%128)*32 + c//128)
    rowmaj = (c_of_e % PB) * GB_ + (c_of_e // PB)
    # c2v DRAM buffer holds slots 2..5 only
    cdrow = (j_of_e - 2) * M + rowmaj          # valid for slots 2..5
    # u/llr DRAM row of a variable = its slot-{0,1} position
    fr_of_e = j_of_e * M + rowmaj              # valid for slots 0..1
    fr_of_v = fr_of_e[ve[:, 0]]                # [N]

    # x-build gathers (dst = parity pi, list pos = check c): variable at
    # (j=pi, c) -> cdram rows of its layer-1 / layer-2 edges
    ix1 = np.empty((2, M), dtype=np.int16)
    ix2 = np.empty((2, M), dtype=np.int16)
    # crossing-2 gathers (dst slot j=2..5, list pos = c): x DRAM row of v(j,c)
    ixu = np.empty((4, M), dtype=np.int16)
    for pi in range(2):
        e = slot_edge[pi]                      # layer-0 edge at (pi, c)
        v = edge_v[e]
        ix1[pi] = cdrow[ve[v, 1]]
        ix2[pi] = cdrow[ve[v, 2]]
    for j in range(2, DC):
        v = edge_v[slot_edge[j]]
        ixu[j - 2] = fr_of_v[v]

    # host llr/output mapping: variable id at each u/llr DRAM row
    vid_of_fr = np.empty(N, dtype=np.int64)
    vid_of_fr[fr_of_v] = np.arange(N)
    return ix1, ix2, ixu, vid_of_fr


def _wrap_idx(idx_m: np.ndarray) -> np.ndarray:
    """dma_gather index layout: list position k at [k%16, k//16],
    replicated across the 8 groups of 16 partitions."""
    w = idx_m.reshape(M // 16, 16).T
    return np.tile(w, (PB // 16, 1)).copy()


def _build_program(alpha: np.ndarray, beta: np.ndarray) -> bacc.Bacc:
    nc = bacc.Bacc(num_swdge_queues=4)

    llr_t = nc.dram_tensor("llr_t", [N, BL], F32, kind="ExternalInput").ap()
    ix1_d = nc.dram_tensor("ix1", [2, PB, M // 16], I16, kind="ExternalInput").ap()
    ix2_d = nc.dram_tensor("ix2", [2, PB, M // 16], I16, kind="ExternalInput").ap()
    ixu_d = nc.dram_tensor("ixu", [4, PB, M // 16], I16, kind="ExternalInput").ap()
    post_d = nc.dram_tensor("post", [2, PB, GB_, BL], F32, kind="ExternalOutput").ap()
    bits_d = nc.dram_tensor("bits", [2, PB, GB_, BL], I32, kind="ExternalOutput").ap()
    # c2v slots 2..5, ping-pong; x1/x2 (v2c messages of layers 1/2 in
    # variable order), ping-pong
    cdrs = [
        nc.dram_tensor("cda", [4 * M, BL], F32).ap(),
        nc.dram_tensor("cdb", [4 * M, BL], F32).ap(),
    ]
    x1rs = [
        nc.dram_tensor("x1a", [N, BL], F32).ap(),
        nc.dram_tensor("x1b", [N, BL], F32).ap(),
    ]
    x2rs = [
        nc.dram_tensor("x2a", [N, BL], F32).ap(),
        nc.dram_tensor("x2b", [N, BL], F32).ap(),
    ]
    cdrv = [c.rearrange("(j p g) e -> j p g e", j=4, p=PB) for c in cdrs]
    x1rv = [u.rearrange("(pi p g) e -> p pi g e", pi=2, p=PB) for u in x1rs]
    x2rv = [u.rearrange("(pi p g) e -> p pi g e", pi=2, p=PB) for u in x2rs]
    bitv = bits_d.rearrange("pi p g e -> p pi g e")

    # SWDGE slot rotation: every gather (prep or not) advances one slot.
    # queue = slot%4 (strict round-robin keeps the 4 ucode queues busy);
    # DMA-completion sem for preps = sems[slot%8], matching Tile's mod-8
    # DMASW lane rotation so each lane pairs with a stable semaphore.
    SW = [0]
    sems = [nc.alloc_semaphore(f"swdge_dma{i}") for i in range(32)]


    def slot():
        q = SW[0] % 4
        s = sems[SW[0] % 32]
        SW[0] += 1
        return q, s

    S1 = CHUNK_BLKS * BL  # free elems per slot per chunk (256)
    # prepare_only+trigger_dma was tried for both gather phases: desc-gen
    # overlapped the check phase, but the generated synchronization raced on
    # hardware (stale gathers) regardless of sem assignment or explicit
    # barriers -- keep the plain self-firing gather path.
    PREP = False
    PREP2 = False

    with tile.TileContext(nc) as tc:
        with (
            tc.tile_pool(name="persist", bufs=1) as pp,
            tc.tile_pool(name="gbp", bufs=4) as gbp,
            tc.tile_pool(name="xdp", bufs=2) as xdp,
            tc.tile_pool(name="wtp", bufs=1) as wtp,
            tc.tile_pool(name="tmp", bufs=1) as tp,
            tc.tile_pool(name="ps", bufs=1, space="PSUM") as psp,
        ):
            ix1 = [pp.tile([PB, M // 16], I16, tag=f"ix1{i}", name=f"ix1{i}") for i in range(2)]
            ix2 = [pp.tile([PB, M // 16], I16, tag=f"ix2{i}", name=f"ix2{i}") for i in range(2)]
            ixu = [pp.tile([PB, M // 16], I16, tag=f"ixu{i}", name=f"ixu{i}") for i in range(4)]
            for i in range(2):
                nc.sync.dma_start(ix1[i][:], ix1_d[i])
                nc.sync.dma_start(ix2[i][:], ix2_d[i])
            for i in range(4):
                nc.sync.dma_start(ixu[i][:], ixu_d[i])

            # hoisted num_idxs registers (a fresh to_reg per gather costs a
            # Pool MOVE each)
            r256 = nc.gpsimd.to_reg(M // 16)
            r512 = nc.gpsimd.to_reg(M // 8)
            r1024 = nc.gpsimd.to_reg(M // 4)
            r2048 = nc.gpsimd.to_reg(M // 2)

            # llr in variable(-row) order, parity-split: [128, 2, 32, 64]
            LV = pp.tile([PB, 2, GB_, BL], F32, tag="lv", name="lv")
            nc.sync.dma_start(
                LV[:], llr_t.rearrange("(pi p g) e -> p pi g e", pi=2, p=PB)
            )
            # x at positions: slots 0,1 (layer 0, variable order) in X0;
            # slots 2..5 (layers 1/2) gathered into U each iteration
            X0 = pp.tile([PB, 2, GB_, BL], F32, tag="x0", name="x0")
            PRE = pp.tile([PB, 2, GB_, BL], F32, tag="pre", name="pre")
            U = pp.tile([PB, 4, GB_, BL], F32, tag="u", name="u")
            # c2v (all 6 slots, check order)
            C = pp.tile([PB, DC, GB_, BL], F32, tag="c", name="c")

            # t=0: x(0) = llr at every edge
            nc.scalar.activation(X0[:], LV[:], ACTF.Copy)
            for b0, nb in ((0, 4), (4, 4), (8, 8), (16, 8), (24, 8)):
                nreg = r512 if nb == 4 else r1024
                for i in range(4):
                    q, _ = slot()
                    nc.gpsimd.dma_gather(
                        U[:, i, b0 : b0 + nb, :],
                        llr_t,
                        ixu[i][:, b0 * 8 : (b0 + nb) * 8],
                        nb * PB, nreg, BL,
                        single_packet=False, queue_num=q,
                    )

            def check_chunk(ck, beta_t, alpha_t, cdvt, last):
                """min-sum check update for chunk ck (CHUNK_BLKS blocks).
                Consumes X0 (slots 0,1) + U (slots 2..5); writes C and DMAs
                slots 2..5 to DRAM."""
                b0 = ck * CHUNK_BLKS
                bs = slice(b0, b0 + CHUNK_BLKS)
                mg = tp.tile([PB, DC, CHUNK_BLKS, BL], F32, tag="mg", name="mg")
                sg = tp.tile([PB, DC, CHUNK_BLKS, BL], F32, tag="sg", name="sg")
                nc.scalar.activation(mg[:, 0:2], X0[:, :, bs, :], ACTF.Abs)
                nc.scalar.activation(mg[:, 2:6], U[:, :, bs, :], ACTF.Abs)
                nc.scalar.activation(sg[:, 0:2], X0[:, :, bs, :], ACTF.Sign)
                nc.scalar.activation(sg[:, 2:6], U[:, :, bs, :], ACTF.Sign)
                pp3 = tp.tile([PB, 3, CHUNK_BLKS, BL], F32, tag="pp3", name="pp3")
                sp3 = tp.tile([PB, 3, CHUNK_BLKS, BL], F32, tag="sp3", name="sp3")
                nc.vector.tensor_tensor(pp3[:], mg[:, 0::2], mg[:, 1::2], ALU.min)
                nc.vector.tensor_tensor(sp3[:], sg[:, 0::2], sg[:, 1::2], ALU.mult)
                # leave-one-pair-out mins
                qq = psp.tile([PB, 3, CHUNK_BLKS, BL], F32, tag="qq", name="qq")
                nc.vector.tensor_tensor(qq[:, 0], pp3[:, 1], pp3[:, 2], ALU.min)
                pv = pp3[:]
                pswap = bass.AP(
                    pv.tensor, pv.offset + 2 * S1,
                    [pv.ap[0], [-S1, 2], [1, S1]],
                )
                p0b = (pp3[:, 0].rearrange("p b e -> p (b e)")[:, None, :]
                       .to_broadcast([PB, 2, S1]))
                nc.vector.tensor_tensor(
                    qq[:, 1:3].rearrange("p a b e -> p a (b e)"), pswap, p0b, ALU.min
                )
                # total sign product * beta
                bsp = psp.tile([PB, CHUNK_BLKS, BL], F32, tag="bsp", name="bsp")
                nc.vector.tensor_tensor(bsp[:], sp3[:, 0], sp3[:, 1], ALU.mult)
                nc.vector.scalar_tensor_tensor(
                    bsp[:], bsp[:], float(beta_t), sp3[:, 2], ALU.mult, ALU.mult
                )
                # leave-one-out min: ex[j] = min(mg[partner(j)], qq[j//2])
                ex = psp.tile([PB, DC, CHUNK_BLKS, BL], F32, tag="ex", name="ex")
                mv = mg[:]
                msw = bass.AP(
                    mv.tensor, mv.offset + S1,
                    [mv.ap[0], [2 * S1, 3], [-S1, 2], [1, S1]],
                )
                qb = (qq[:].rearrange("p a b e -> p a (b e)")[:, :, None, :]
                      .to_broadcast([PB, 3, 2, S1]))
                nc.vector.tensor_tensor(
                    ex[:].rearrange("p (a b) c e -> p a b (c e)", a=3), msw, qb, ALU.min
                )
                # c2v = (sign * beta*sprod) * exclmin
                bb = bsp[:, None, :, :].to_broadcast([PB, DC, CHUNK_BLKS, BL])
                nc.vector.tensor_tensor(sg[:], sg[:], bb, ALU.mult)
                nc.vector.tensor_tensor(C[:, :, bs, :], sg[:], ex[:], ALU.mult)
                for j in range(2, DC):
                    nc.sync.dma_start(cdvt[j - 2][:, bs, :], C[:, j, bs, :])

            for t in range(T):
                beta_t = float(beta[t])
                alpha_t = float(alpha[t])
                cdt, cdvt = cdrs[t % 2], cdrv[t % 2]
                cdvt_full = cdvt
                x1t, x1vt = x1rs[t % 2], x1rv[t % 2]
                x2t, x2vt = x2rs[t % 2], x2rv[t % 2]
                last = t == T - 1

                # --- check phase (DVE/ACT; Pool desc-gens the preps below) ---
                for ck in range(NCHUNK):
                    check_chunk(ck, beta_t, alpha_t, cdvt, last)

                # --- gm/gh preps: c2v of layers 1/2 -> variable order.
                # desc-gen runs during the check phase; the triggers wait for
                # the c2v DRAM writes. ---
                # groups (start block, nblocks, tile slot, in-tile offset):
                # the two small trailing groups share tile 3's halves, so 5
                # gather groups fit the 4-deep gm/gh pool without aliasing
                GGRPS = ((0, 4, 0, 0), (4, 4, 0, 4), (8, 8, 1, 0),
                         (16, 8, 2, 0), (24, 4, 3, 0), (28, 4, 3, 4))
                tiles = [
                    (gbp.tile([PB, 2, 8, BL], F32, tag="gm", name=f"gm{t}_{k}"),
                     gbp.tile([PB, 2, 8, BL], F32, tag="gh", name=f"gh{t}_{k}"))
                    for k in range(4)
                ]
                for b0, nb, ti, o in GGRPS:
                    ls = slice(b0 * 8, (b0 + nb) * 8)
                    nreg = r512 if nb == 4 else r1024
                    gm, gh = tiles[ti]
                    for pi in range(2):
                        q, s = slot()
                        nc.gpsimd.dma_gather(
                            gh[:, pi, o : o + nb, :], cdt, ix2[pi][:, ls],
                            nb * PB, nreg, BL,
                            single_packet=False, queue_num=q,
                            prepare_only=PREP, sem=s if PREP else None,
                        )
                    for pi in range(2):
                        q, s = slot()
                        nc.gpsimd.dma_gather(
                            gm[:, pi, o : o + nb, :], cdt, ix1[pi][:, ls],
                            nb * PB, nreg, BL,
                            single_packet=False, queue_num=q,
                            prepare_only=PREP, sem=s if PREP else None,
                        )
                if PREP:
                    # barrier: echo-read one element from every c2v chunk
                    # write's range, then a Pool op consuming the echo tile
                    # -- the triggers (in-order on Pool) thereby fire only
                    # after all 32 c2v writes have landed in DRAM.
                    echo = tp.tile([PB, 4, 8, 1], F32, tag="echo", name=f"echo{t}")
                    for j in range(4):
                        nc.sync.dma_start(
                            echo[:, j], cdvt_full[j][:, ::CHUNK_BLKS, 0:1]
                        )
                    scr = tp.tile([PB, 32], F32, tag="scr", name=f"scr{t}")
                    nc.gpsimd.partition_broadcast(
                        scr[:], echo[:].rearrange("p j g e -> p (j g e)")
                    )
                    for q in range(4):
                        nc.gpsimd.trigger_dma(
                            count=None, queue_num=q,
                            signals_writable=[scr[:, q : q + 1]],
                        )

                # pre = llr + alpha*C0 (gather-independent): the DVE does
                # this while the first gm/gh gathers are still in flight
                if not last:
                    nc.vector.scalar_tensor_tensor(
                        PRE[:], C[:, 0:2, :, :], alpha_t, LV[:], ALU.mult, ALU.add
                    )

                # --- window: x build (or posterior on the last iteration) ---
                for b0, nb, ti, o in GGRPS:
                    hs = slice(b0, b0 + nb)
                    gm = tiles[ti][0][:, :, o : o + nb, :]
                    gh = tiles[ti][1][:, :, o : o + nb, :]
                    lvh = LV[:, :, hs, :]
                    c0h = C[:, 0:2, hs, :]
                    w1f = wtp.tile([PB, 2, 8, BL], F32, tag="w1", name="w1")
                    w1 = w1f[:, :, 0:nb, :]
                    if last:
                        # posterior = llr + (C0 + gm + gh); bits = post < 0
                        nc.vector.tensor_tensor(w1, gm, gh, ALU.add)
                        nc.vector.tensor_tensor(w1, w1, c0h, ALU.add)
                        nc.vector.tensor_tensor(w1, w1, lvh, ALU.add)
                        btf = xdp.tile([PB, 2, 8, BL], I32, tag="xd1", name="bt")
                        nc.vector.tensor_scalar(
                            btf[:, :, 0:nb, :], w1, 0.0, None, ALU.is_lt
                        )
                        for pi in range(2):
                            nc.sync.dma_start(
                                post_d[pi][:, hs, :], w1f[:, pi, 0:nb, :]
                            )
                        nc.sync.dma_start(bitv[:, :, hs, :], btf[:, :, 0:nb, :])
                    else:
                        # x_l1 = pre + a*gh -> x1 DRAM; x_l2 = pre + a*gm -> x2
                        # x_l0 = llr + a*(gm+gh) -> X0 (next check phase)
                        xd1 = xdp.tile([PB, 2, 8, BL], F32, tag="xd1", name="xd1")
                        xd2 = xdp.tile([PB, 2, 8, BL], F32, tag="xd2", name="xd2")
                        # per-parity ops: each fires as soon as its single
                        # gather lands instead of waiting for the pair
                        for pi in range(2):
                            nc.vector.scalar_tensor_tensor(
                                xd1[:, pi, 0:nb, :], gh[:, pi], alpha_t,
                                PRE[:, pi, hs, :], ALU.mult, ALU.add
                            )
                            nc.sync.dma_start(
                                x1vt[:, pi, hs, :], xd1[:, pi, 0:nb, :]
                            )
                        for pi in range(2):
                            nc.vector.scalar_tensor_tensor(
                                xd2[:, pi, 0:nb, :], gm[:, pi], alpha_t,
                                PRE[:, pi, hs, :], ALU.mult, ALU.add
                            )
                            nc.sync.dma_start(
                                x2vt[:, pi, hs, :], xd2[:, pi, 0:nb, :]
                            )
                        nc.vector.tensor_tensor(w1, gm, gh, ALU.add)
                        nc.vector.scalar_tensor_tensor(
                            X0[:, :, hs, :], w1, alpha_t, lvh, ALU.mult, ALU.add
                        )

                if not last:
                    # --- crossing 2 preps: x -> position order, slots 2..5.
                    # desc-gen overlaps the gm/gh transfers + window math;
                    # triggers wait for the x DRAM writes. ---
                    for b0, nb in ((0, 4), (4, 4), (8, 8), (16, 8), (24, 8)):
                        gs = slice(b0 * 8, (b0 + nb) * 8)
                        ds = slice(b0, b0 + nb)
                        nreg = r512 if nb == 4 else r1024
                        for i in range(4):
                            q, s = slot()
                            nc.gpsimd.dma_gather(
                                U[:, i, ds, :],
                                x1t if i < 2 else x2t,
                                ixu[i][:, gs],
                                nb * PB, nreg, BL,
                                single_packet=False, queue_num=q,
                                prepare_only=PREP2, sem=s if PREP2 else None,
                            )
                    if PREP2:
                        echo2 = tp.tile([PB, 2, 2, 8, 1], F32, tag="echo2", name=f"echo2_{t}")
                        for pi in range(2):
                            nc.sync.dma_start(
                                echo2[:, 0, pi], x1vt[:, pi, ::CHUNK_BLKS, 0:1]
                            )
                            nc.sync.dma_start(
                                echo2[:, 1, pi], x2vt[:, pi, ::CHUNK_BLKS, 0:1]
                            )
                        scr2 = tp.tile([PB, 32], F32, tag="scr", name=f"scr2_{t}")
                        nc.gpsimd.partition_broadcast(
                            scr2[:], echo2[:].rearrange("p a b g e -> p (a b g e)")
                        )
                        for q in range(4):
                            nc.gpsimd.trigger_dma(
                                count=None, queue_num=q,
                                signals_writable=[scr2[:, q : q + 1]],
                            )

    nc.compile()
    return nc


def _prepare(llr, edge_v, edge_c, beta, alpha):
    ix1, ix2, ixu, vid_of_fr = _derive_graph(edge_v, edge_c)
    ix1w = np.stack([_wrap_idx(ix1[i]) for i in range(2)])
    ix2w = np.stack([_wrap_idx(ix2[i]) for i in range(2)])
    ixuw = np.stack([_wrap_idx(ixu[i]) for i in range(4)])

    llr = np.asarray(llr, dtype=np.float32)
    in_maps = []
    for k in range(NCORES):
        llr_t = np.ascontiguousarray(llr[k * BL : (k + 1) * BL, vid_of_fr].T)
        in_maps.append({"llr_t": llr_t, "ix1": ix1w, "ix2": ix2w, "ixu": ixuw})
    return in_maps, vid_of_fr


def _assemble(results, vid_of_fr):
    posterior = np.empty((B, N), dtype=np.float32)
    bits = np.empty((B, N), dtype=np.int32)
    for k in range(NCORES):
        pd = results[k]["post"].reshape(N, BL)  # row = pi*4096 + p*32 + g
        bd = results[k]["bits"].reshape(N, BL)
        posterior[k * BL : (k + 1) * BL, vid_of_fr] = pd.T
        bits[k * BL : (k + 1) * BL, vid_of_fr] = bd.T
    return bits, posterior


def _run(llr, edge_v, edge_c, beta, alpha, trace=False, tmpdir=None):
    in_maps, vid_of_fr = _prepare(llr, edge_v, edge_c, beta, alpha)
    nc = _build_program(np.asarray(alpha, np.float32), np.asarray(beta, np.float32))
    res = run_bass_kernel_spmd(
        nc, in_maps, list(range(NCORES)), trace=trace, tmpdir=tmpdir
    )
    return _assemble(res.results, vid_of_fr), res


def kernel(llr, edge_v, edge_c, beta, alpha):
    (bits, posterior), _ = _run(llr, edge_v, edge_c, beta, alpha, trace=False)
    return bits, posterior
